# revision 1
# baseline (speedup 1.0000x reference)
"""BinaryLinear Trainium2 kernel.

Computes out = x @ sign(weight).T + bias for x [8192, 4096] f32,
weight [4096, 4096] f32, bias [4096] f32.

Strategy: data-parallel over the token dim across 8 NeuronCores
(1024 tokens per core, weight/bias replicated, no collectives).

Per-core pipeline (no DRAM scratch):
  1. x tiles [128t, 4096i] are cast f32->bf16 during the SWDGE DMA load,
     then one whole-tile XBAR transpose SBUF->SBUF lands each in
     XT [128i, 32k, 1024t] (8 transposes total for x).
  2. weight rows likewise: cast to bf16 SBUF slabs [128o, 4096i]; one
     XBAR transpose per slab fills WT_n [128i, 32k, 512o] (4 per output
     block); sign() is applied in place on the Scalar engine (scale=1e30
     pushes tiny values off the LUT's zero neighborhood; sign(0)=0
     preserved). Few, large transposes minimize XBAR-mode serialization
     against other DMA traffic.
  3. TensorE: psum[m] += XT[k,m].T @ WT[n,k], fp32 accumulation in PSUM
     over all 32 k-tiles; 8 token tiles <-> 8 PSUM banks.
  4. DVE adds the (partition-broadcast) bias while copying PSUM->SBUF;
     HWDGE stores f32 output tiles.
"""

import numpy as np

import concourse.mybir as mybir
import concourse.tile as tile
from concourse import bacc
from concourse.bass import ts

P = 128
TOKENS, IN_F, OUT_F = 8192, 4096, 4096
N_CORES = 8
N_TILE = 512   # output-feature block (one PSUM bank of f32)

F32 = mybir.dt.float32
BF16 = mybir.dt.bfloat16


def build_nc(t_shard=TOKENS // N_CORES, in_f=IN_F, out_f=OUT_F, repeat=1):
    m_tiles = t_shard // P      # token tiles of 128
    n_tiles = out_f // N_TILE   # output blocks of 512
    ko_tiles = in_f // P        # k tiles of 128
    j_tiles = N_TILE // P       # 128-row slabs per output block

    nc = bacc.Bacc(None, target_bir_lowering=False, debug=False)

    x = nc.dram_tensor("x", [t_shard, in_f], F32, kind="ExternalInput")
    w = nc.dram_tensor("weight", [out_f, in_f], F32, kind="ExternalInput")
    b = nc.dram_tensor("bias", [out_f], F32, kind="ExternalInput")
    out = nc.dram_tensor("out", [t_shard, out_f], F32, kind="ExternalOutput")

    with tile.TileContext(nc) as tc:
        with (
            tc.tile_pool(name="consts", bufs=2) as const_pool,
            tc.tile_pool(name="stage", bufs=6) as stage_pool,
            tc.tile_pool(name="xt", bufs=1) as xt_pool,
            tc.tile_pool(name="wt", bufs=4) as wt_pool,
            tc.tile_pool(name="out_sb", bufs=3) as out_pool,
            tc.tile_pool(name="ps", bufs=8, space="PSUM") as psum_pool,
        ):
          for _rep in range(repeat):

            def cast_slab(src_rows):
                """SWDGE cast f32->bf16 of 128 DRAM rows into SBUF."""
                slab = stage_pool.tile([P, in_f], BF16, name="slab", tag="stage")
                nc.gpsimd.dma_start(slab, src_rows)
                return slab

            half_k = max(1, ko_tiles // 2)  # k-tiles per wt half-tile

            def emit_wt(n):
                """Build signed WT half-tiles [128i, 16k, 512o] for block n.

                Two halves (same total SBUF as one whole-block tile at
                bufs=4 vs 2) double the pipeline depth and release PSUM
                of the producer chain at half-block granularity.
                """
                slabs = [
                    cast_slab(w[ts(n * j_tiles + j, P), :])
                    for j in range(j_tiles)
                ]
                halves = []
                for h in range(ko_tiles // half_k):
                    wt_h = wt_pool.tile(
                        [P, half_k, N_TILE], BF16, name=f"wt_{n}_{h}", tag="wt"
                    )
                    for j in range(j_tiles):
                        # NOTE: must stay on nc.sync — issuing these
                        # transposes on nc.scalar's HWDGE ring corrupts
                        # results on HW (passes CoreSim, rel err 0.42).
                        nc.sync.dma_start(
                            wt_h[:, :, ts(j, P)],
                            slabs[j][:, ts(h, half_k * P)],
                            transpose=True,
                        )
                    # sign in place; scale pushes tiny magnitudes off the
                    # LUT's zero breakpoint while keeping sign(0) == 0
                    nc.scalar.activation(
                        wt_h, wt_h, mybir.ActivationFunctionType.Sign,
                        scale=1.0e30,
                    )
                    halves.append(wt_h)
                return halves

            # ---- head: W block 0 first (it gates the first matmuls),
            # then x tiles (each transposed whole so matmuls can start
            # after the first).
            wts = {0: emit_wt(0)}
            xt_all = xt_pool.tile(
                [P, ko_tiles, t_shard], BF16, name="xt_all", tag="xt"
            )
            for m in range(m_tiles):
                slab = cast_slab(x[ts(m, P), :])
                nc.sync.dma_start(
                    xt_all[:, :, ts(m, P)], slab, transpose=True
                )
                if m == 0 and n_tiles > 1:
                    wts[1] = emit_wt(1)

            # ---- main loop over output blocks
            for n in range(n_tiles):
                if n + 2 < n_tiles:
                    wts[n + 2] = emit_wt(n + 2)
                wt_n = wts.pop(n)

                # per-block bias, replicated across partitions via DMA
                bias_rep = const_pool.tile(
                    [P, N_TILE], F32, name="bias_rep", tag="bias"
                )
                nc.gpsimd.dma_start(
                    bias_rep,
                    b[None, ts(n, N_TILE)].broadcast_to([P, N_TILE]),
                )

                # two phase-shifted groups of 4 PSUM banks: group B's
                # matmuls overlap group A's output copies
                half = max(1, m_tiles // 2)
                for g0 in range(0, m_tiles, half):
                    ms = range(g0, min(g0 + half, m_tiles))
                    psums = {
                        m: psum_pool.tile(
                            [P, N_TILE], F32, name=f"ps_{n}_{m}", tag="ps"
                        )
                        for m in ms
                    }
                    for k in range(ko_tiles):
                        for m in ms:
                            nc.tensor.matmul(
                                psums[m],
                                xt_all[:, k, ts(m, P)],
                                wt_n[k // half_k][:, k % half_k, :],
                                start=(k == 0),
                                stop=(k == ko_tiles - 1),
                            )
                    for m in ms:
                        out_sb = out_pool.tile(
                            [P, N_TILE], F32, name="out_sb", tag="out_sb"
                        )
                        nc.vector.tensor_tensor(
                            out_sb, psums[m], bias_rep, mybir.AluOpType.add
                        )
                        # SWDGE for stores keeps the sync HWDGE ring
                        # nearly transpose-only (fewer FIFO stalls)
                        nc.gpsimd.dma_start(
                            out[ts(m, P), ts(n, N_TILE)], out_sb
                        )

    nc.compile()
    return nc


_NC_CACHE = {}


def _get_nc(shape_key):
    if shape_key not in _NC_CACHE:
        _NC_CACHE[shape_key] = build_nc(*shape_key)
    return _NC_CACHE[shape_key]


def kernel(x, weight, bias, _trace=False):
    from concourse.bass_utils import run_bass_kernel_spmd

    x = np.ascontiguousarray(np.asarray(x, dtype=np.float32))
    weight = np.ascontiguousarray(np.asarray(weight, dtype=np.float32))
    bias = np.ascontiguousarray(np.asarray(bias, dtype=np.float32))

    tokens = x.shape[0]
    t_shard = tokens // N_CORES
    nc = _get_nc((t_shard, x.shape[1], weight.shape[0]))

    in_maps = [
        {
            "x": x[c * t_shard : (c + 1) * t_shard],
            "weight": weight,
            "bias": bias,
        }
        for c in range(N_CORES)
    ]
    res = run_bass_kernel_spmd(
        nc, in_maps, core_ids=list(range(N_CORES)), trace=_trace
    )
    out = np.concatenate([r["out"] for r in res.results], axis=0)
    if _trace:
        return out, res
    return out



# revision 2
# speedup vs baseline: 1.0607x; 1.0607x over previous
"""BinaryLinear Trainium2 kernel (v2 — weight-stationary).

Computes out = x @ sign(weight).T + bias for x [8192, 4096] f32,
weight [4096, 4096] f32, bias [4096] f32.

Strategy: data-parallel over the token dim across 8 NeuronCores
(1024 tokens per core, weight/bias replicated, no collectives).

v2 rationale: in v1 the stationary operand (x tile) changed every
matmul, exposing LDWEIGHTS and limiting TensorE to ~60% of the bf16
roofline. Here the weight tile wt[k,n] [128i, 128o] is the stationary
operand, reused across two 512-token moving streams, so each
LDWEIGHTS (~53-107ns) hides under 2x213ns of streaming. Output lands
as out_T [4096o, 1024t] per core (PSUM partitions = outputs); the
host transposes during the unshard/gather step.

Per-core pipeline (no DRAM scratch):
  1. x tiles [128t, 4096i] are cast f32->bf16 during the SWDGE DMA
     load, then one whole-tile XBAR transpose SBUF->SBUF lands each in
     XT [128i, 32k, 1024t] (8 transposes for x), resident all kernel.
  2. weight rows likewise: per 128-output tile n, cast rows to a bf16
     slab [128o, 4096i]; one XBAR transpose fills WT_n [128i, 32k,
     128o]; sign() in place on ScalarE (scale=1e30 pushes tiny values
     off the LUT's zero neighborhood; sign(0)=0 preserved).
  3. TensorE per (n, k): stationary WT_n[:,k,:], two matmuls stream
     XT[:,k,0:512] and XT[:,k,512:1024] into psum banks [128o, 512t],
     fp32 accumulation over all 32 k.
  4. DVE tensor_scalar adds the per-partition bias while copying
     PSUM->SBUF; SWDGE stores f32 [128o, 1024t] rows of out_T.
"""

import numpy as np

import concourse.mybir as mybir
import concourse.tile as tile
from concourse import bacc
from concourse.bass import ts

P = 128
TOKENS, IN_F, OUT_F = 8192, 4096, 4096
N_CORES = 8

F32 = mybir.dt.float32
BF16 = mybir.dt.bfloat16


def build_nc(t_shard=TOKENS // N_CORES, in_f=IN_F, out_f=OUT_F, repeat=1):
    m_tiles = t_shard // P      # token tiles of 128
    n_tiles = out_f // P        # output tiles of 128
    ko_tiles = in_f // P        # k tiles of 128
    t_half = t_shard // 2       # moving-stream length per matmul

    nc = bacc.Bacc(None, target_bir_lowering=False, debug=False)

    x = nc.dram_tensor("x", [t_shard, in_f], F32, kind="ExternalInput")
    w = nc.dram_tensor("weight", [out_f, in_f], F32, kind="ExternalInput")
    b = nc.dram_tensor("bias", [out_f], F32, kind="ExternalInput")
    # transposed output: rows = out features, cols = this core's tokens
    out = nc.dram_tensor("out", [out_f, t_shard], F32, kind="ExternalOutput")

    with tile.TileContext(nc) as tc:
        with (
            tc.tile_pool(name="stage", bufs=4) as stage_pool,
            tc.tile_pool(name="xt", bufs=1) as xt_pool,
            tc.tile_pool(name="wt", bufs=4) as wt_pool,
            tc.tile_pool(name="bias", bufs=4) as bias_pool,
            tc.tile_pool(name="out_sb", bufs=4) as out_pool,
            tc.tile_pool(name="ps", bufs=8, space="PSUM") as psum_pool,
        ):
          for _rep in range(repeat):

            def cast_slab(src_rows):
                """SWDGE cast f32->bf16 of 128 DRAM rows into SBUF."""
                slab = stage_pool.tile([P, in_f], BF16, name="slab", tag="stage")
                nc.gpsimd.dma_start(slab, src_rows)
                return slab

            def emit_wt(n):
                """Signed WT tile [128i, 32k, 128o] for output tile n."""
                slab = cast_slab(w[ts(n, P), :])
                wt_n = wt_pool.tile(
                    [P, ko_tiles, P], BF16, name=f"wt_{n}", tag="wt"
                )
                # NOTE: transposes must stay on nc.sync — issuing them on
                # nc.scalar's HWDGE ring corrupts results on HW.
                nc.sync.dma_start(wt_n, slab, transpose=True)
                nc.scalar.activation(
                    wt_n, wt_n, mybir.ActivationFunctionType.Sign,
                    scale=1.0e30,
                )
                bias_n = bias_pool.tile([P, 1], F32, name=f"b_{n}", tag="bias")
                nc.gpsimd.dma_start(bias_n, b[ts(n, P), None])
                return wt_n, bias_n

            # ---- head: weight tile 0 first (gates the first matmuls),
            # then x tiles, then weight tile 1.
            wts = {0: emit_wt(0)}
            xt_all = xt_pool.tile(
                [P, ko_tiles, t_shard], BF16, name="xt_all", tag="xt"
            )
            for m in range(m_tiles):
                slab = cast_slab(x[ts(m, P), :])
                nc.sync.dma_start(
                    xt_all[:, :, ts(m, P)], slab, transpose=True
                )
                if m == 0 and n_tiles > 1:
                    wts[1] = emit_wt(1)

            # ---- main loop over output tiles
            for n in range(n_tiles):
                if n + 2 < n_tiles:
                    wts[n + 2] = emit_wt(n + 2)
                wt_n, bias_n = wts.pop(n)

                psums = [
                    psum_pool.tile([P, t_half], F32, name=f"ps_{n}_{h}", tag="ps")
                    for h in range(2)
                ]
                for k in range(ko_tiles):
                    for h in range(2):
                        nc.tensor.matmul(
                            psums[h],
                            wt_n[:, k, :],
                            xt_all[:, k, ts(h, t_half)],
                            start=(k == 0),
                            stop=(k == ko_tiles - 1),
                        )
                out_sb = out_pool.tile(
                    [P, t_shard], F32, name="out_sb", tag="out_sb"
                )
                for h in range(2):
                    nc.vector.tensor_scalar(
                        out_sb[:, ts(h, t_half)], psums[h], bias_n, None,
                        mybir.AluOpType.add,
                    )
                nc.gpsimd.dma_start(out[ts(n, P), :], out_sb)

    nc.compile()
    return nc


_NC_CACHE = {}


def _get_nc(shape_key):
    if shape_key not in _NC_CACHE:
        _NC_CACHE[shape_key] = build_nc(*shape_key)
    return _NC_CACHE[shape_key]


def kernel(x, weight, bias, _trace=False):
    from concourse.bass_utils import run_bass_kernel_spmd

    x = np.ascontiguousarray(np.asarray(x, dtype=np.float32))
    weight = np.ascontiguousarray(np.asarray(weight, dtype=np.float32))
    bias = np.ascontiguousarray(np.asarray(bias, dtype=np.float32))

    tokens = x.shape[0]
    t_shard = tokens // N_CORES
    nc = _get_nc((t_shard, x.shape[1], weight.shape[0]))

    in_maps = [
        {
            "x": x[c * t_shard : (c + 1) * t_shard],
            "weight": weight,
            "bias": bias,
        }
        for c in range(N_CORES)
    ]
    res = run_bass_kernel_spmd(
        nc, in_maps, core_ids=list(range(N_CORES)), trace=_trace
    )
    # per-core result is out_T [out_f, t_shard]; transpose during gather
    out = np.concatenate(
        [np.ascontiguousarray(r["out"].T) for r in res.results], axis=0
    )
    if _trace:
        return out, res
    return out
